# revision 2
# baseline (speedup 1.0000x reference)
"""Causal self-attention Trainium2 Bass/Tile kernel.

Problem: B=4, T=2048, C=2048, H=16 heads, d=128. fp32 I/O.
Sharding over 8 cores: core i -> (batch b = i//2, head-group g = i%2).
Each core computes attention + partial c_proj for its 8 heads on its
batch; host sums the two head-group partials per batch.

Per-core math (HL = 8 local heads, ch = HL*d = 1024 local channels):
  V[t, ch]   = sum_c x[t, c] Wv[ch, c]          (A0)
  qT[dd, t]  = sum_c Wq[dd, c] x[t, c]          (A1, per head)
  kT[dd, t]  = sum_c Wk[dd, c] x[t, c]
  S[tq, tk]  = sum_dd qT[dd, tq] kT[dd, tk]     (B, causal row blocks)
  P          = exp(S / sqrt(d) + mask)           (no max-sub: logits bounded)
  attn[tq,d] = (sum_tk P[tq,tk] V[tk,d]) / l[tq]
  y[t, o]    = sum_ch attnT[ch, t] WpT[ch, o]   (C, partial over local ch)

All matmuls bf16 inputs, fp32 PSUM accumulation.
"""

import math
import sys

import numpy as np

sys.path.insert(0, "/opt/trn_rl_repo")

import ml_dtypes  # noqa: E402

import concourse.bass as bass  # noqa: E402
import concourse.bacc as bacc  # noqa: E402
import concourse.mybir as mybir  # noqa: E402
import concourse.tile as tile  # noqa: E402
from concourse.masks import make_identity  # noqa: E402

BF16 = mybir.dt.bfloat16
F32 = mybir.dt.float32
P = 128
SG = 512  # psum bank width in fp32


def _chunks(total, size):
    out = []
    o = 0
    while o < total:
        w = min(size, total - o)
        out.append((o, w))
        o += w
    return out


def build_nc(T=2048, C=2048, HL=8, d=128):
    """Build the per-core Bass program (SPMD: same program on all cores)."""
    TB = T // P  # token blocks
    CB = C // P  # contraction chunks over C
    CH = HL * d  # local channels
    scale = 1.0 / math.sqrt(d)
    QH = min(1024, T)  # qk psum tile width
    YH = min(1024, C)  # proj psum tile width

    nc = bacc.Bacc(None, target_bir_lowering=False)

    xT = nc.dram_tensor("xT", [C, T], BF16, kind="ExternalInput")
    # wqk[p, cc, h, 0:d]=WqT chunk, [.., d:2d]=WkT chunk  (c = cc*128 + p)
    wqk = nc.dram_tensor("wqk", [P, CB, HL, 2 * d], BF16, kind="ExternalInput")
    wv = nc.dram_tensor("wv", [C, CH], BF16, kind="ExternalInput")
    wp = nc.dram_tensor("wp", [CH, C], BF16, kind="ExternalInput")
    y = nc.dram_tensor("y", [T, C], F32, kind="ExternalOutput")

    with tile.TileContext(nc) as tc:
        with (
            tc.tile_pool(name="const", bufs=1) as constp,
            tc.tile_pool(name="attn", bufs=HL) as attnp,
            tc.tile_pool(name="vsb", bufs=TB) as vpool,
        ):
            ident = constp.tile([P, P], BF16)
            make_identity(nc, ident)
            # additive causal mask: 0 where tk <= tq, -1e9 where tk > tq
            cmask = constp.tile([P, P], F32)
            nc.gpsimd.memset(cmask[:], -1e9)
            nc.gpsimd.affine_select(
                out=cmask[:],
                in_=cmask[:],
                compare_op=mybir.AluOpType.is_gt,
                fill=0.0,
                base=0,
                pattern=[[1, P]],
                channel_multiplier=-1,
            )

            attnTs = []
            Vs = []

            with tc.tile_pool(name="xt", bufs=CB) as xtp:
                xts = []
                for c in range(CB):
                    xt = xtp.tile([P, T], BF16)
                    nc.sync.dma_start(xt[:], xT[c * P : (c + 1) * P, :])
                    xts.append(xt)

                # ---------------- Phase A0: V = x @ Wv.T ----------------
                with (
                    tc.tile_pool(name="wv", bufs=CB) as wvp,
                    tc.tile_pool(name="psv", bufs=2, space="PSUM") as psv,
                ):
                    wvts = []
                    for c in range(CB):
                        wvt = wvp.tile([P, CH], BF16)
                        nc.sync.dma_start(wvt[:], wv[c * P : (c + 1) * P, :])
                        wvts.append(wvt)
                    for m in range(TB):
                        ps = psv.tile([P, CH], F32, tag="psv")
                        for c in range(CB):
                            for o, w in _chunks(CH, SG):
                                nc.tensor.matmul(
                                    ps[:, o : o + w],
                                    xts[c][:, m * P : (m + 1) * P],
                                    wvts[c][:, o : o + w],
                                    start=(c == 0),
                                    stop=(c == CB - 1),
                                )
                        V = vpool.tile([P, CH], BF16, tag="V")
                        nc.scalar.copy(V[:], ps[:])
                        Vs.append(V)

                # ------------- Phases A1 + B: per-head qk + attention ----
                with (
                    tc.tile_pool(name="wqk", bufs=2) as wqkp,
                    tc.tile_pool(name="qk", bufs=4) as qkp,
                    tc.tile_pool(name="prow", bufs=2) as prp,
                    tc.tile_pool(name="pt", bufs=TB + 2) as ptp,
                    tc.tile_pool(name="lr", bufs=8) as lrp,
                    tc.tile_pool(name="atmp", bufs=2) as atp,
                    tc.tile_pool(name="pss", bufs=2, space="PSUM") as pss,
                    tc.tile_pool(name="pst", bufs=2, space="PSUM") as pst,
                    tc.tile_pool(name="psp", bufs=2, space="PSUM") as psp,
                ):
                    for h in range(HL):
                        wqt = wqkp.tile([P, CB, 2 * d], BF16, tag="wqk")
                        nc.sync.dma_start(wqt[:], wqk[:, :, h, :])

                        qT = qkp.tile([P, T], BF16, tag="qT")
                        kT = qkp.tile([P, T], BF16, tag="kT")
                        for mi, dst in ((0, qT), (1, kT)):
                            for ho, hw in _chunks(T, QH):
                                ps = pss.tile([P, QH], F32, tag="s")
                                for c in range(CB):
                                    for o, w in _chunks(hw, SG):
                                        nc.tensor.matmul(
                                            ps[:, o : o + w],
                                            wqt[:, c, mi * d : (mi + 1) * d],
                                            xts[c][:, ho + o : ho + o + w],
                                            start=(c == 0),
                                            stop=(c == CB - 1),
                                        )
                                nc.scalar.copy(dst[:, ho : ho + hw], ps[:, :hw])

                        attnT = attnp.tile([P, T], BF16, tag="attnT")
                        for i in range(TB):
                            ncol = (i + 1) * P
                            groups = _chunks(ncol, SG)
                            prow = prp.tile([P, T], BF16, tag="prow")
                            lparts = lrp.tile([P, max(len(groups), 2)], F32, tag="lp")
                            for gi, (o, w) in enumerate(groups):
                                ps = pss.tile([P, QH], F32, tag="s")
                                nc.tensor.matmul(
                                    ps[:, :w],
                                    qT[:, i * P : (i + 1) * P],
                                    kT[:, o : o + w],
                                    start=True,
                                    stop=True,
                                )
                                if o <= i * P < o + w:  # diagonal block
                                    dc = i * P - o
                                    nc.vector.tensor_add(
                                        ps[:, dc : dc + P],
                                        ps[:, dc : dc + P],
                                        cmask[:],
                                    )
                                nc.scalar.activation(
                                    prow[:, o : o + w],
                                    ps[:, :w],
                                    mybir.ActivationFunctionType.Exp,
                                    scale=scale,
                                    accum_out=lparts[:, gi : gi + 1],
                                )
                            rr = lrp.tile([P, 1], F32, tag="rr")
                            ll = lrp.tile([P, 1], F32, tag="ll")
                            nc.vector.reduce_sum(
                                ll[:],
                                lparts[:, : len(groups)],
                                axis=mybir.AxisListType.X,
                            )
                            nc.vector.reciprocal(rr[:], ll[:])

                            pts = []
                            for j in range(i + 1):
                                tp = pst.tile([P, P], BF16, tag="tr")
                                nc.tensor.transpose(
                                    tp[:], prow[:, j * P : (j + 1) * P], ident[:]
                                )
                                pt = ptp.tile([P, P], BF16, tag="pt")
                                if j % 2 == 0:
                                    nc.scalar.copy(pt[:], tp[:])
                                else:
                                    nc.vector.tensor_copy(pt[:], tp[:])
                                pts.append(pt)

                            po = psp.tile([P, P], F32, tag="pv")
                            for j in range(i + 1):
                                nc.tensor.matmul(
                                    po[:],
                                    pts[j][:],
                                    Vs[j][:, h * d : (h + 1) * d],
                                    start=(j == 0),
                                    stop=(j == i),
                                )
                            atmp = atp.tile([P, P], BF16, tag="atmp")
                            nc.vector.tensor_scalar_mul(atmp[:], po[:], rr[:])
                            tp2 = pst.tile([P, P], BF16, tag="tr")
                            nc.tensor.transpose(tp2[:], atmp[:], ident[:])
                            nc.scalar.copy(attnT[:, i * P : (i + 1) * P], tp2[:])
                        attnTs.append(attnT)

            # ---------------- Phase C: y = attn @ Wp.T (partial) --------
            with (
                tc.tile_pool(name="wp", bufs=CH // P) as wpp,
                tc.tile_pool(name="ysb", bufs=2) as ysbp,
                tc.tile_pool(name="psy", bufs=2, space="PSUM") as psy,
            ):
                wpts = []
                for ch in range(CH // P):
                    wpt = wpp.tile([P, C], BF16, tag="wp")
                    nc.sync.dma_start(wpt[:], wp[ch * P : (ch + 1) * P, :])
                    wpts.append(wpt)
                for tb in range(TB):
                    ysb = ysbp.tile([P, C], F32, tag="ysb")
                    for ho, hw in _chunks(C, YH):
                        ps = psy.tile([P, YH], F32, tag="y")
                        for ch in range(CH // P):
                            hd = ch * P // d  # owning local head of this chunk
                            for o, w in _chunks(hw, SG):
                                nc.tensor.matmul(
                                    ps[:, o : o + w],
                                    attnTs[hd][:, tb * P : (tb + 1) * P],
                                    wpts[ch][:, ho + o : ho + o + w],
                                    start=(ch == 0),
                                    stop=(ch == CH // P - 1),
                                )
                        nc.vector.tensor_copy(ysb[:, ho : ho + hw], ps[:, :hw])
                    nc.sync.dma_start(y[tb * P : (tb + 1) * P, :], ysb[:])

    return nc


def make_core_inputs(x, W_attn, W_proj, b, g, T=2048, C=2048, HL=8, d=128):
    """Host-side shard + layout prep for core (batch b, head-group g)."""
    bf16 = ml_dtypes.bfloat16
    CB = C // P
    CH = HL * d
    xb = np.asarray(x[b], dtype=np.float32)  # [T, C]
    xT = np.ascontiguousarray(xb.T).astype(bf16)  # [C, T]

    q_rows = W_attn[g * CH : (g + 1) * CH, :]  # [CH, C]
    k_rows = W_attn[C + g * CH : C + (g + 1) * CH, :]
    v_rows = W_attn[2 * C + g * CH : 2 * C + (g + 1) * CH, :]

    # wqk[p, cc, h, col]: col 0:d -> WqT, d:2d -> WkT; c = cc*128 + p
    wqkT = np.empty((C, HL, 2 * d), dtype=np.float32)
    for h in range(HL):
        wqkT[:, h, :d] = q_rows[h * d : (h + 1) * d, :].T
        wqkT[:, h, d:] = k_rows[h * d : (h + 1) * d, :].T
    wqk = np.ascontiguousarray(
        wqkT.reshape(CB, P, HL, 2 * d).transpose(1, 0, 2, 3)
    ).astype(bf16)

    wv = np.ascontiguousarray(v_rows.T).astype(bf16)  # [C, CH]
    wpm = np.ascontiguousarray(W_proj[:, g * CH : (g + 1) * CH].T).astype(
        bf16
    )  # [CH, C]
    return {"xT": xT, "wqk": wqk, "wv": wv, "wp": wpm}


def core_reference(xT, wqk, wv, wp, T=2048, C=2048, HL=8, d=128):
    """Numpy replica of one core's program (bf16 inputs, fp32 accum)."""
    CB = C // P
    xTf = np.asarray(xT, dtype=np.float32)
    x = xTf.T  # [T, C]
    wqkf = np.asarray(wqk, dtype=np.float32).transpose(1, 0, 2, 3).reshape(C, HL, 2 * d)
    V = x @ np.asarray(wv, dtype=np.float32)  # [T, CH]
    out = np.zeros((T, HL * d), dtype=np.float32)
    for h in range(HL):
        q = x @ wqkf[:, h, :d]  # [T, d]
        k = x @ wqkf[:, h, d:]
        S = (q @ k.T) / math.sqrt(d)
        mask = np.triu(np.ones((T, T), dtype=bool), 1)
        S = np.where(mask, -np.inf, S)
        Pm = np.exp(S)
        Pm = Pm / Pm.sum(-1, keepdims=True)
        out[:, h * d : (h + 1) * d] = Pm @ V[:, h * d : (h + 1) * d]
    return out @ np.asarray(wp, dtype=np.float32)  # [T, C] partial


_CACHE = {}


def _get_nc():
    if "nc" not in _CACHE:
        nc = build_nc()
        nc.compile()
        _CACHE["nc"] = nc
    return _CACHE["nc"]


def run_cores(in_maps, trace=False):
    from concourse.bass_utils import run_bass_kernel_spmd

    nc = _get_nc()
    return run_bass_kernel_spmd(nc, in_maps, list(range(len(in_maps))), trace=trace)


def kernel(x, W_attn, W_proj):
    x = np.asarray(x, dtype=np.float32)
    W_attn = np.asarray(W_attn, dtype=np.float32)
    W_proj = np.asarray(W_proj, dtype=np.float32)
    B = x.shape[0]
    in_maps = [make_core_inputs(x, W_attn, W_proj, i // 2, i % 2) for i in range(8)]
    res = run_cores(in_maps).results
    y = np.stack(
        [res[2 * b]["y"].astype(np.float32) + res[2 * b + 1]["y"] for b in range(B)]
    )
    return y


# revision 4
# speedup vs baseline: 22.0189x; 22.0189x over previous
"""Causal self-attention Trainium2 Bass/Tile kernel.

Problem: B=4, T=2048, C=2048, H=16 heads, d=128. fp32 I/O.
Sharding over 8 cores: core i -> (batch b = i//2, head-group g = i%2).
Each core computes attention + partial c_proj for its 8 heads on its
batch; host sums the two head-group partials per batch.

Per-core math (HL = 8 local heads, ch = HL*d = 1024 local channels):
  V[t, ch]   = sum_c x[t, c] Wv[ch, c]          (A0)
  qT[dd, t]  = sum_c Wq[dd, c] x[t, c]          (A1, per head)
  kT[dd, t]  = sum_c Wk[dd, c] x[t, c]
  S[tq, tk]  = sum_dd qT[dd, tq] kT[dd, tk]     (B, causal row blocks)
  P          = exp(S / sqrt(d) + mask)           (no max-sub: logits bounded)
  attn[tq,d] = (sum_tk P[tq,tk] V[tk,d]) / l[tq]
  y[t, o]    = sum_ch attnT[ch, t] WpT[ch, o]   (C, partial over local ch)

All matmuls bf16 inputs, fp32 PSUM accumulation.
"""

import math
import sys

import numpy as np

sys.path.insert(0, "/opt/trn_rl_repo")

import ml_dtypes  # noqa: E402

import concourse.bass as bass  # noqa: E402
import concourse.bacc as bacc  # noqa: E402
import concourse.mybir as mybir  # noqa: E402
import concourse.tile as tile  # noqa: E402
from concourse.masks import make_identity  # noqa: E402

BF16 = mybir.dt.bfloat16
F32 = mybir.dt.float32
P = 128
SG = 512  # psum bank width in fp32


def _chunks(total, size):
    out = []
    o = 0
    while o < total:
        w = min(size, total - o)
        out.append((o, w))
        o += w
    return out


def build_nc(T=2048, C=2048, HL=8, d=128, reps=1):
    """Build the per-core Bass program (SPMD: same program on all cores).

    reps > 1 wraps the whole body in a device-side loop (timing only).
    """
    TB = T // P  # token blocks
    CB = C // P  # contraction chunks over C
    CH = HL * d  # local channels
    scale = 1.0 / math.sqrt(d)
    QH = min(1024, T)  # qk psum tile width
    YH = min(1024, C)  # proj psum tile width

    nc = bacc.Bacc(None, target_bir_lowering=False)

    xT = nc.dram_tensor("xT", [C, T], BF16, kind="ExternalInput")
    # wqk[p, cc, h, 0:d]=WqT chunk, [.., d:2d]=WkT chunk  (c = cc*128 + p)
    wqk = nc.dram_tensor("wqk", [P, CB, HL, 2 * d], BF16, kind="ExternalInput")
    wv = nc.dram_tensor("wv", [C, CH], BF16, kind="ExternalInput")
    wp = nc.dram_tensor("wp", [CH, C], BF16, kind="ExternalInput")
    y = nc.dram_tensor("y", [T, C], F32, kind="ExternalOutput")

    import contextlib

    with tile.TileContext(nc) as tc, contextlib.ExitStack() as es:
        if reps > 1:
            es.enter_context(tc.For_i(0, reps, 1))
        with (
            tc.tile_pool(name="const", bufs=1) as constp,
            tc.tile_pool(name="attn", bufs=HL) as attnp,
            tc.tile_pool(name="vsb", bufs=TB) as vpool,
        ):
            ident = constp.tile([P, P], BF16)
            make_identity(nc, ident)
            # additive causal mask: 0 where tk <= tq, -1e9 where tk > tq
            cmask = constp.tile([P, P], F32)
            nc.gpsimd.memset(cmask[:], -1e9)
            nc.gpsimd.affine_select(
                out=cmask[:],
                in_=cmask[:],
                compare_op=mybir.AluOpType.is_gt,
                fill=0.0,
                base=0,
                pattern=[[1, P]],
                channel_multiplier=-1,
            )

            attnTs = []
            Vs = []

            with tc.tile_pool(name="xt", bufs=CB) as xtp:
                xts = []
                for c in range(CB):
                    xt = xtp.tile([P, T], BF16)
                    nc.sync.dma_start(xt[:], xT[c * P : (c + 1) * P, :])
                    xts.append(xt)

                # ---------------- Phase A0: V = x @ Wv.T ----------------
                with (
                    tc.tile_pool(name="wv", bufs=CB) as wvp,
                    tc.tile_pool(name="psv", bufs=2, space="PSUM") as psv,
                ):
                    wvts = []
                    for c in range(CB):
                        wvt = wvp.tile([P, CH], BF16)
                        nc.sync.dma_start(wvt[:], wv[c * P : (c + 1) * P, :])
                        wvts.append(wvt)
                    for m in range(TB):
                        ps = psv.tile([P, CH], F32, tag="psv")
                        for c in range(CB):
                            for o, w in _chunks(CH, SG):
                                nc.tensor.matmul(
                                    ps[:, o : o + w],
                                    xts[c][:, m * P : (m + 1) * P],
                                    wvts[c][:, o : o + w],
                                    start=(c == 0),
                                    stop=(c == CB - 1),
                                )
                        V = vpool.tile([P, CH], BF16, tag="V")
                        nc.scalar.copy(V[:], ps[:])
                        Vs.append(V)

                # ------------- Phases A1 + B: per-head qk + attention ----
                with (
                    tc.tile_pool(name="wqk", bufs=2) as wqkp,
                    tc.tile_pool(name="qk", bufs=4) as qkp,
                    tc.tile_pool(name="prow", bufs=2) as prp,
                    tc.tile_pool(name="pt", bufs=TB + 2) as ptp,
                    tc.tile_pool(name="lr", bufs=8) as lrp,
                    tc.tile_pool(name="atmp", bufs=2) as atp,
                    tc.tile_pool(name="pss", bufs=2, space="PSUM") as pss,
                    tc.tile_pool(name="pst", bufs=2, space="PSUM") as pst,
                    tc.tile_pool(name="psp", bufs=2, space="PSUM") as psp,
                ):
                    for h in range(HL):
                        wqt = wqkp.tile([P, CB, 2 * d], BF16, tag="wqk")
                        nc.sync.dma_start(wqt[:], wqk[:, :, h, :])

                        qT = qkp.tile([P, T], BF16, tag="qT")
                        kT = qkp.tile([P, T], BF16, tag="kT")
                        for mi, dst in ((0, qT), (1, kT)):
                            for ho, hw in _chunks(T, QH):
                                ps = pss.tile([P, QH], F32, tag="s")
                                for c in range(CB):
                                    for o, w in _chunks(hw, SG):
                                        nc.tensor.matmul(
                                            ps[:, o : o + w],
                                            wqt[:, c, mi * d : (mi + 1) * d],
                                            xts[c][:, ho + o : ho + o + w],
                                            start=(c == 0),
                                            stop=(c == CB - 1),
                                        )
                                nc.scalar.copy(dst[:, ho : ho + hw], ps[:, :hw])

                        attnT = attnp.tile([P, T], BF16, tag="attnT")
                        for i in range(TB):
                            ncol = (i + 1) * P
                            groups = _chunks(ncol, SG)
                            prow = prp.tile([P, T], BF16, tag="prow")
                            lparts = lrp.tile([P, max(len(groups), 2)], F32, tag="lp")
                            for gi, (o, w) in enumerate(groups):
                                ps = pss.tile([P, QH], F32, tag="s")
                                nc.tensor.matmul(
                                    ps[:, :w],
                                    qT[:, i * P : (i + 1) * P],
                                    kT[:, o : o + w],
                                    start=True,
                                    stop=True,
                                )
                                if o <= i * P < o + w:  # diagonal block
                                    dc = i * P - o
                                    nc.vector.tensor_add(
                                        ps[:, dc : dc + P],
                                        ps[:, dc : dc + P],
                                        cmask[:],
                                    )
                                nc.scalar.activation(
                                    prow[:, o : o + w],
                                    ps[:, :w],
                                    mybir.ActivationFunctionType.Exp,
                                    scale=scale,
                                    accum_out=lparts[:, gi : gi + 1],
                                )
                            rr = lrp.tile([P, 1], F32, tag="rr")
                            ll = lrp.tile([P, 1], F32, tag="ll")
                            nc.vector.reduce_sum(
                                ll[:],
                                lparts[:, : len(groups)],
                                axis=mybir.AxisListType.X,
                            )
                            nc.vector.reciprocal(rr[:], ll[:])

                            pts = []
                            for j in range(i + 1):
                                tp = pst.tile([P, P], BF16, tag="tr")
                                nc.tensor.transpose(
                                    tp[:], prow[:, j * P : (j + 1) * P], ident[:]
                                )
                                pt = ptp.tile([P, P], BF16, tag="pt")
                                if j % 2 == 0:
                                    nc.scalar.copy(pt[:], tp[:])
                                else:
                                    nc.vector.tensor_copy(pt[:], tp[:])
                                pts.append(pt)

                            po = psp.tile([P, P], F32, tag="pv")
                            for j in range(i + 1):
                                nc.tensor.matmul(
                                    po[:],
                                    pts[j][:],
                                    Vs[j][:, h * d : (h + 1) * d],
                                    start=(j == 0),
                                    stop=(j == i),
                                )
                            atmp = atp.tile([P, P], BF16, tag="atmp")
                            nc.vector.tensor_scalar_mul(atmp[:], po[:], rr[:])
                            tp2 = pst.tile([P, P], BF16, tag="tr")
                            nc.tensor.transpose(tp2[:], atmp[:], ident[:])
                            nc.scalar.copy(attnT[:, i * P : (i + 1) * P], tp2[:])
                        attnTs.append(attnT)

            # ---------------- Phase C: y = attn @ Wp.T (partial) --------
            with (
                tc.tile_pool(name="wp", bufs=CH // P) as wpp,
                tc.tile_pool(name="ysb", bufs=2) as ysbp,
                tc.tile_pool(name="psy", bufs=2, space="PSUM") as psy,
            ):
                wpts = []
                for ch in range(CH // P):
                    wpt = wpp.tile([P, C], BF16, tag="wp")
                    nc.sync.dma_start(wpt[:], wp[ch * P : (ch + 1) * P, :])
                    wpts.append(wpt)
                for tb in range(TB):
                    ysb = ysbp.tile([P, C], F32, tag="ysb")
                    for ho, hw in _chunks(C, YH):
                        ps = psy.tile([P, YH], F32, tag="y")
                        for ch in range(CH // P):
                            hd = ch * P // d  # owning local head of this chunk
                            for o, w in _chunks(hw, SG):
                                nc.tensor.matmul(
                                    ps[:, o : o + w],
                                    attnTs[hd][:, tb * P : (tb + 1) * P],
                                    wpts[ch][:, ho + o : ho + o + w],
                                    start=(ch == 0),
                                    stop=(ch == CH // P - 1),
                                )
                        nc.vector.tensor_copy(ysb[:, ho : ho + hw], ps[:, :hw])
                    nc.sync.dma_start(y[tb * P : (tb + 1) * P, :], ysb[:])

    return nc


def make_core_inputs(x, W_attn, W_proj, b, g, T=2048, C=2048, HL=8, d=128):
    """Host-side shard + layout prep for core (batch b, head-group g)."""
    bf16 = ml_dtypes.bfloat16
    CB = C // P
    CH = HL * d
    xb = np.asarray(x[b], dtype=np.float32)  # [T, C]
    xT = np.ascontiguousarray(xb.T).astype(bf16)  # [C, T]

    q_rows = W_attn[g * CH : (g + 1) * CH, :]  # [CH, C]
    k_rows = W_attn[C + g * CH : C + (g + 1) * CH, :]
    v_rows = W_attn[2 * C + g * CH : 2 * C + (g + 1) * CH, :]

    # wqk[p, cc, h, col]: col 0:d -> WqT, d:2d -> WkT; c = cc*128 + p
    wqkT = np.empty((C, HL, 2 * d), dtype=np.float32)
    for h in range(HL):
        wqkT[:, h, :d] = q_rows[h * d : (h + 1) * d, :].T
        wqkT[:, h, d:] = k_rows[h * d : (h + 1) * d, :].T
    wqk = np.ascontiguousarray(
        wqkT.reshape(CB, P, HL, 2 * d).transpose(1, 0, 2, 3)
    ).astype(bf16)

    wv = np.ascontiguousarray(v_rows.T).astype(bf16)  # [C, CH]
    wpm = np.ascontiguousarray(W_proj[:, g * CH : (g + 1) * CH].T).astype(
        bf16
    )  # [CH, C]
    return {"xT": xT, "wqk": wqk, "wv": wv, "wp": wpm}


def core_reference(xT, wqk, wv, wp, T=2048, C=2048, HL=8, d=128):
    """Numpy replica of one core's program (bf16 inputs, fp32 accum)."""
    CB = C // P
    xTf = np.asarray(xT, dtype=np.float32)
    x = xTf.T  # [T, C]
    wqkf = np.asarray(wqk, dtype=np.float32).transpose(1, 0, 2, 3).reshape(C, HL, 2 * d)
    V = x @ np.asarray(wv, dtype=np.float32)  # [T, CH]
    out = np.zeros((T, HL * d), dtype=np.float32)
    for h in range(HL):
        q = x @ wqkf[:, h, :d]  # [T, d]
        k = x @ wqkf[:, h, d:]
        S = (q @ k.T) / math.sqrt(d)
        mask = np.triu(np.ones((T, T), dtype=bool), 1)
        S = np.where(mask, -np.inf, S)
        Pm = np.exp(S)
        Pm = Pm / Pm.sum(-1, keepdims=True)
        out[:, h * d : (h + 1) * d] = Pm @ V[:, h * d : (h + 1) * d]
    return out @ np.asarray(wp, dtype=np.float32)  # [T, C] partial


_CACHE = {}


def _get_nc():
    if "nc" not in _CACHE:
        nc = build_nc()
        nc.compile()
        _CACHE["nc"] = nc
    return _CACHE["nc"]


def run_cores(in_maps, trace=False):
    from concourse.bass_utils import run_bass_kernel_spmd

    nc = _get_nc()
    return run_bass_kernel_spmd(nc, in_maps, list(range(len(in_maps))), trace=trace)


def kernel(x, W_attn, W_proj):
    x = np.asarray(x, dtype=np.float32)
    W_attn = np.asarray(W_attn, dtype=np.float32)
    W_proj = np.asarray(W_proj, dtype=np.float32)
    B = x.shape[0]
    in_maps = [make_core_inputs(x, W_attn, W_proj, i // 2, i % 2) for i in range(8)]
    res = run_cores(in_maps).results
    y = np.stack(
        [res[2 * b]["y"].astype(np.float32) + res[2 * b + 1]["y"] for b in range(B)]
    )
    return y


# revision 10
# speedup vs baseline: 28.1925x; 1.2804x over previous
"""Causal self-attention Trainium2 Bass/Tile kernel.

Problem: B=4, T=2048, C=2048, H=16 heads, d=128. fp32 I/O.
Sharding over 8 cores: core i -> (batch b = i//2, head-group g = i%2).
Each core computes attention + partial c_proj for its 8 heads on its
batch; host sums the two head-group partials per batch.

Per-core math (HL = 8 local heads, ch = HL*d = 1024 local channels):
  V[t, ch]   = sum_c x[t, c] Wv[ch, c]          (A0)
  qT[dd, t]  = sum_c Wq[dd, c] x[t, c]          (A1, per head)
  kT[dd, t]  = sum_c Wk[dd, c] x[t, c]
  S[tq, tk]  = sum_dd qT[dd, tq] kT[dd, tk]     (B, causal row blocks)
  P          = exp(S / sqrt(d) + mask)           (no max-sub: logits bounded)
  attn[tq,d] = (sum_tk P[tq,tk] V[tk,d]) / l[tq]
  y[t, o]    = sum_ch attnT[ch, t] WpT[ch, o]   (C, partial over local ch)

All matmuls bf16 inputs, fp32 PSUM accumulation.
"""

import math
import sys

import numpy as np

sys.path.insert(0, "/opt/trn_rl_repo")

import ml_dtypes  # noqa: E402

import concourse.bass as bass  # noqa: E402
import concourse.bacc as bacc  # noqa: E402
import concourse.mybir as mybir  # noqa: E402
import concourse.tile as tile  # noqa: E402
from concourse.masks import make_identity  # noqa: E402

BF16 = mybir.dt.bfloat16
F32 = mybir.dt.float32
P = 128
SG = 512  # psum bank width in fp32


def _chunks(total, size):
    out = []
    o = 0
    while o < total:
        w = min(size, total - o)
        out.append((o, w))
        o += w
    return out


def build_nc(T=2048, C=2048, HL=8, d=128, reps=1):
    """Build the per-core Bass program (SPMD: same program on all cores).

    reps > 1 wraps the whole body in a device-side loop (timing only).
    """
    TB = T // P  # token blocks
    CB = C // P  # contraction chunks over C
    CH = HL * d  # local channels
    scale = 1.0 / math.sqrt(d)
    QH = min(1024, T)  # qk psum tile width
    YH = min(1024, C)  # proj psum tile width

    nc = bacc.Bacc(None, target_bir_lowering=False)

    xT = nc.dram_tensor("xT", [C, T], BF16, kind="ExternalInput")
    # wqk[p, cc, h, 0:d]=WqT chunk, [.., d:2d]=WkT chunk  (c = cc*128 + p)
    wqk = nc.dram_tensor("wqk", [P, CB, HL, 2 * d], BF16, kind="ExternalInput")
    wv = nc.dram_tensor("wv", [C, CH], BF16, kind="ExternalInput")
    wp = nc.dram_tensor("wp", [CH, C], BF16, kind="ExternalInput")
    y = nc.dram_tensor("y", [T, C], F32, kind="ExternalOutput")

    import contextlib

    with tile.TileContext(nc) as tc, contextlib.ExitStack() as es:
        if reps > 1:
            es.enter_context(tc.For_i(0, reps, 1))
        with (
            tc.tile_pool(name="const", bufs=1) as constp,
            tc.tile_pool(name="attn", bufs=HL) as attnp,
            tc.tile_pool(name="vsb", bufs=TB) as vpool,
        ):
            # ones column-block for row-sum matmuls: L = ones.T @ P
            ones = constp.tile([P, P], BF16)
            nc.gpsimd.memset(ones[:], 1.0)
            # additive causal mask for St[tk, tq] diag: -1e9 where tk > tq
            cmask = constp.tile([P, P], F32)
            nc.gpsimd.memset(cmask[:], -1e9)
            nc.gpsimd.affine_select(
                out=cmask[:],
                in_=cmask[:],
                compare_op=mybir.AluOpType.is_gt,
                fill=0.0,
                base=0,
                pattern=[[-1, P]],
                channel_multiplier=1,
            )

            attnTs = []
            Vs = []

            with tc.tile_pool(name="xt", bufs=CB) as xtp:
                xts = []
                for c in range(CB):
                    xt = xtp.tile([P, T], BF16)
                    nc.sync.dma_start(xt[:], xT[c * P : (c + 1) * P, :])
                    xts.append(xt)

                # ---------------- Phase A0: V = x @ Wv.T ----------------
                with (
                    tc.tile_pool(name="wv", bufs=CB) as wvp,
                    tc.tile_pool(name="psv", bufs=2, space="PSUM") as psv,
                ):
                    wvts = []
                    for c in range(CB):
                        wvt = wvp.tile([P, CH], BF16)
                        nc.sync.dma_start(wvt[:], wv[c * P : (c + 1) * P, :])
                        wvts.append(wvt)
                    for m in range(TB):
                        ps = psv.tile([P, CH], F32, tag="psv")
                        for c in range(CB):
                            for o, w in _chunks(CH, SG):
                                nc.tensor.matmul(
                                    ps[:, o : o + w],
                                    xts[c][:, m * P : (m + 1) * P],
                                    wvts[c][:, o : o + w],
                                    start=(c == 0),
                                    stop=(c == CB - 1),
                                )
                        V = vpool.tile([P, CH], BF16, tag="V")
                        nc.scalar.copy(V[:], ps[:])
                        Vs.append(V)

                # ------------- Phases A1 + B: per-head qk + attention ----
                QW = min(SG, T)  # tq strip width for PV/L accumulation
                with (
                    tc.tile_pool(name="wqk", bufs=2) as wqkp,
                    tc.tile_pool(name="qk", bufs=4) as qkp,
                    tc.tile_pool(name="pt", bufs=TB + 2) as ptp,
                    tc.tile_pool(name="rl", bufs=2) as rlp,
                    tc.tile_pool(name="psqk", bufs=1, space="PSUM") as psqk,
                    tc.tile_pool(name="pss", bufs=2, space="PSUM") as pss,
                    tc.tile_pool(name="psa", bufs=2, space="PSUM") as psa,
                    tc.tile_pool(name="psl", bufs=2, space="PSUM") as psl,
                ):
                    for h in range(HL):
                        wqt = wqkp.tile([P, CB, 2 * d], BF16, tag="wqk")
                        nc.sync.dma_start(wqt[:], wqk[:, :, h, :])

                        qT = qkp.tile([P, T], BF16, tag="qT")
                        kT = qkp.tile([P, T], BF16, tag="kT")
                        for mi, dst in ((0, qT), (1, kT)):
                            for ho, hw in _chunks(T, QH):
                                ps = psqk.tile([P, QH], F32, tag="qk")
                                for c in range(CB):
                                    for o, w in _chunks(hw, SG):
                                        nc.tensor.matmul(
                                            ps[:, o : o + w],
                                            wqt[:, c, mi * d : (mi + 1) * d],
                                            xts[c][:, ho + o : ho + o + w],
                                            start=(c == 0),
                                            stop=(c == CB - 1),
                                        )
                                nc.scalar.copy(dst[:, ho : ho + hw], ps[:, :hw])

                        # --- attention: St[tk, tq] orientation, per tq strip
                        attnT = attnp.tile([P, T], BF16, tag="attnT")
                        for q4 in range(T // QW):
                            s0 = q4 * QW  # strip cols [s0, s1)
                            s1 = s0 + QW
                            njs = s1 // P  # j blocks feeding this strip
                            pa = psa.tile([P, QW], F32, tag="pa")
                            pl = psl.tile([P, QW], F32, tag="pl")
                            pts = []
                            for j in range(njs):
                                c0 = max(j * P, s0)
                                w = s1 - c0
                                ps = pss.tile([P, QW], F32, tag="st")
                                nc.tensor.matmul(
                                    ps[:, :w],
                                    kT[:, j * P : (j + 1) * P],
                                    qT[:, c0:s1],
                                    start=True,
                                    stop=True,
                                )
                                if j * P >= s0:  # diagonal block lives here
                                    nc.vector.tensor_add(
                                        ps[:, 0:P], ps[:, 0:P], cmask[:]
                                    )
                                pt = ptp.tile([P, QW], BF16, tag="pt")
                                nc.scalar.activation(
                                    pt[:, :w],
                                    ps[:, :w],
                                    mybir.ActivationFunctionType.Exp,
                                    scale=scale,
                                )
                                pts.append((pt, c0, w))
                            for j, (pt, c0, w) in enumerate(pts):
                                nc.tensor.matmul(
                                    pa[:, c0 - s0 : s1 - s0],
                                    Vs[j][:, h * d : (h + 1) * d],
                                    pt[:, :w],
                                    start=(j == 0),
                                    stop=(j == njs - 1),
                                    skip_group_check=True,
                                )
                            for j, (pt, c0, w) in enumerate(pts):
                                nc.tensor.matmul(
                                    pl[:, c0 - s0 : s1 - s0],
                                    ones[:],
                                    pt[:, :w],
                                    start=(j == 0),
                                    stop=(j == njs - 1),
                                    skip_group_check=True,
                                )
                            rl = rlp.tile([P, QW], F32, tag="rl")
                            nc.vector.reciprocal(rl[:], pl[:])
                            nc.vector.tensor_tensor(
                                out=attnT[:, s0:s1],
                                in0=pa[:],
                                in1=rl[:],
                                op=mybir.AluOpType.mult,
                            )
                        attnTs.append(attnT)

            # ---------------- Phase C: y = attn @ Wp.T (partial) --------
            with (
                tc.tile_pool(name="wp", bufs=CH // P) as wpp,
                tc.tile_pool(name="ysb", bufs=2) as ysbp,
                tc.tile_pool(name="psy", bufs=2, space="PSUM") as psy,
            ):
                wpts = []
                for ch in range(CH // P):
                    wpt = wpp.tile([P, C], BF16, tag="wp")
                    nc.sync.dma_start(wpt[:], wp[ch * P : (ch + 1) * P, :])
                    wpts.append(wpt)
                for tb in range(TB):
                    ysb = ysbp.tile([P, C], F32, tag="ysb")
                    for ho, hw in _chunks(C, YH):
                        ps = psy.tile([P, YH], F32, tag="y")
                        for ch in range(CH // P):
                            hd = ch * P // d  # owning local head of this chunk
                            for o, w in _chunks(hw, SG):
                                nc.tensor.matmul(
                                    ps[:, o : o + w],
                                    attnTs[hd][:, tb * P : (tb + 1) * P],
                                    wpts[ch][:, ho + o : ho + o + w],
                                    start=(ch == 0),
                                    stop=(ch == CH // P - 1),
                                )
                        nc.vector.tensor_copy(ysb[:, ho : ho + hw], ps[:, :hw])
                    nc.sync.dma_start(y[tb * P : (tb + 1) * P, :], ysb[:])

    return nc


def make_core_inputs(x, W_attn, W_proj, b, g, T=2048, C=2048, HL=8, d=128):
    """Host-side shard + layout prep for core (batch b, head-group g)."""
    bf16 = ml_dtypes.bfloat16
    CB = C // P
    CH = HL * d
    xb = np.asarray(x[b], dtype=np.float32)  # [T, C]
    xT = np.ascontiguousarray(xb.T).astype(bf16)  # [C, T]

    q_rows = W_attn[g * CH : (g + 1) * CH, :]  # [CH, C]
    k_rows = W_attn[C + g * CH : C + (g + 1) * CH, :]
    v_rows = W_attn[2 * C + g * CH : 2 * C + (g + 1) * CH, :]

    # wqk[p, cc, h, col]: col 0:d -> WqT, d:2d -> WkT; c = cc*128 + p
    wqkT = np.empty((C, HL, 2 * d), dtype=np.float32)
    for h in range(HL):
        wqkT[:, h, :d] = q_rows[h * d : (h + 1) * d, :].T
        wqkT[:, h, d:] = k_rows[h * d : (h + 1) * d, :].T
    wqk = np.ascontiguousarray(
        wqkT.reshape(CB, P, HL, 2 * d).transpose(1, 0, 2, 3)
    ).astype(bf16)

    wv = np.ascontiguousarray(v_rows.T).astype(bf16)  # [C, CH]
    wpm = np.ascontiguousarray(W_proj[:, g * CH : (g + 1) * CH].T).astype(
        bf16
    )  # [CH, C]
    return {"xT": xT, "wqk": wqk, "wv": wv, "wp": wpm}


def core_reference(xT, wqk, wv, wp, T=2048, C=2048, HL=8, d=128):
    """Numpy replica of one core's program (bf16 inputs, fp32 accum)."""
    CB = C // P
    xTf = np.asarray(xT, dtype=np.float32)
    x = xTf.T  # [T, C]
    wqkf = np.asarray(wqk, dtype=np.float32).transpose(1, 0, 2, 3).reshape(C, HL, 2 * d)
    V = x @ np.asarray(wv, dtype=np.float32)  # [T, CH]
    out = np.zeros((T, HL * d), dtype=np.float32)
    for h in range(HL):
        q = x @ wqkf[:, h, :d]  # [T, d]
        k = x @ wqkf[:, h, d:]
        S = (q @ k.T) / math.sqrt(d)
        mask = np.triu(np.ones((T, T), dtype=bool), 1)
        S = np.where(mask, -np.inf, S)
        Pm = np.exp(S)
        Pm = Pm / Pm.sum(-1, keepdims=True)
        out[:, h * d : (h + 1) * d] = Pm @ V[:, h * d : (h + 1) * d]
    return out @ np.asarray(wp, dtype=np.float32)  # [T, C] partial


_CACHE = {}


def _get_nc():
    if "nc" not in _CACHE:
        nc = build_nc()
        nc.compile()
        _CACHE["nc"] = nc
    return _CACHE["nc"]


def run_cores(in_maps, trace=False):
    from concourse.bass_utils import run_bass_kernel_spmd

    nc = _get_nc()
    return run_bass_kernel_spmd(nc, in_maps, list(range(len(in_maps))), trace=trace)


def kernel(x, W_attn, W_proj):
    x = np.asarray(x, dtype=np.float32)
    W_attn = np.asarray(W_attn, dtype=np.float32)
    W_proj = np.asarray(W_proj, dtype=np.float32)
    B = x.shape[0]
    in_maps = [make_core_inputs(x, W_attn, W_proj, i // 2, i % 2) for i in range(8)]
    res = run_cores(in_maps).results
    y = np.stack(
        [res[2 * b]["y"].astype(np.float32) + res[2 * b + 1]["y"] for b in range(B)]
    )
    return y


# revision 18
# speedup vs baseline: 33.9170x; 1.2031x over previous
"""Causal self-attention Trainium2 Bass/Tile kernel.

Problem: B=4, T=2048, C=2048, H=16 heads, d=128. fp32 I/O.
Sharding over 8 cores: core i -> (batch b = i//2, head-group g = i%2).
Each core computes attention + partial c_proj for its 8 heads on its
batch; host sums the two head-group partials per batch.

Per-core math (HL = 8 local heads, ch = HL*d = 1024 local channels):
  V[t, ch]   = sum_c x[t, c] Wv[ch, c]          (A0)
  qT[dd, t]  = sum_c Wq[dd, c] x[t, c]          (A1, per head)
  kT[dd, t]  = sum_c Wk[dd, c] x[t, c]
  S[tq, tk]  = sum_dd qT[dd, tq] kT[dd, tk]     (B, causal row blocks)
  P          = exp(S / sqrt(d) + mask)           (no max-sub: logits bounded)
  attn[tq,d] = (sum_tk P[tq,tk] V[tk,d]) / l[tq]
  y[t, o]    = sum_ch attnT[ch, t] WpT[ch, o]   (C, partial over local ch)

All matmuls bf16 inputs, fp32 PSUM accumulation.
"""

import math
import sys

import numpy as np

sys.path.insert(0, "/opt/trn_rl_repo")

import ml_dtypes  # noqa: E402

import concourse.bass as bass  # noqa: E402
import concourse.bacc as bacc  # noqa: E402
import concourse.mybir as mybir  # noqa: E402
import concourse.tile as tile  # noqa: E402
from concourse.masks import make_identity  # noqa: E402

BF16 = mybir.dt.bfloat16
F32 = mybir.dt.float32
P = 128
SG = 512  # psum bank width in fp32


def _chunks(total, size):
    out = []
    o = 0
    while o < total:
        w = min(size, total - o)
        out.append((o, w))
        o += w
    return out


def build_nc(T=2048, C=2048, HL=8, d=128, reps=1):
    """Build the per-core Bass program (SPMD: same program on all cores).

    reps > 1 wraps the whole body in a device-side loop (timing only).
    """
    TB = T // P  # token blocks
    CB = C // P  # contraction chunks over C
    CH = HL * d  # local channels
    scale = 1.0 / math.sqrt(d)
    QH = min(1024, T)  # qk psum tile width
    YH = min(1024, C)  # proj psum tile width

    nc = bacc.Bacc(None, target_bir_lowering=False)

    xT = nc.dram_tensor("xT", [C, T], BF16, kind="ExternalInput")
    # wqk[p, cc, h, 0:d]=WqT chunk, [.., d:2d]=WkT chunk  (c = cc*128 + p)
    wqk = nc.dram_tensor("wqk", [P, CB, HL, 2 * d], BF16, kind="ExternalInput")
    wv = nc.dram_tensor("wv", [C, CH], BF16, kind="ExternalInput")
    wp = nc.dram_tensor("wp", [CH, C], BF16, kind="ExternalInput")
    y = nc.dram_tensor("y", [T, C], F32, kind="ExternalOutput")

    import contextlib

    with tile.TileContext(nc) as tc, contextlib.ExitStack() as es:
        if reps > 1:
            es.enter_context(tc.For_i(0, reps, 1))
        with (
            tc.tile_pool(name="const", bufs=1) as constp,
            tc.tile_pool(name="attn", bufs=HL) as attnp,
            tc.tile_pool(name="vsb", bufs=TB) as vpool,
        ):
            # ones column-block for row-sum matmuls: L = ones.T @ P
            ones = constp.tile([P, P], BF16)
            nc.gpsimd.memset(ones[:], 1.0)
            # additive causal mask for St[tk, tq] diag: -1e9 where tk > tq
            cmask = constp.tile([P, P], F32)
            nc.gpsimd.memset(cmask[:], -1e9)
            nc.gpsimd.affine_select(
                out=cmask[:],
                in_=cmask[:],
                compare_op=mybir.AluOpType.is_gt,
                fill=0.0,
                base=0,
                pattern=[[-1, P]],
                channel_multiplier=1,
            )

            attnTs = []
            Vs = []

            with tc.tile_pool(name="xt", bufs=CB) as xtp:
                # ---------------- Phase A0: V = x @ Wv.T ----------------
                with (
                    tc.tile_pool(name="wv", bufs=CB) as wvp,
                    tc.tile_pool(name="psv", bufs=2, space="PSUM") as psv,
                ):
                    wvts = []
                    for c in range(CB):
                        wvt = wvp.tile([P, CH], BF16)
                        nc.sync.dma_start(wvt[:], wv[c * P : (c + 1) * P, :])
                        wvts.append(wvt)
                    # xt after wv, in column quarters, so A0's first
                    # m-blocks can start before the whole of x lands
                    xts = [
                        xtp.tile([P, T], BF16, name=f"xt{c}", tag="xt")
                        for c in range(CB)
                    ]
                    for qo, qw in _chunks(T, max(T // 4, P)):
                        for c in range(CB):
                            nc.sync.dma_start(
                                xts[c][:, qo : qo + qw],
                                xT[c * P : (c + 1) * P, qo : qo + qw],
                            )
                    for m in range(TB):
                        ps = psv.tile([P, CH], F32, tag="psv")
                        for c in range(CB):
                            for o, w in _chunks(CH, SG):
                                nc.tensor.matmul(
                                    ps[:, o : o + w],
                                    xts[c][:, m * P : (m + 1) * P],
                                    wvts[c][:, o : o + w],
                                    start=(c == 0),
                                    stop=(c == CB - 1),
                                )
                        V = vpool.tile([P, CH], BF16, tag="V")
                        nc.scalar.copy(V[:], ps[:])
                        Vs.append(V)

                # ------------- Phases A1 + B: per-head qk + attention ----
                QW = min(SG, T)  # tq strip width for PV/L accumulation
                with (
                    tc.tile_pool(name="wqk", bufs=2) as wqkp,
                    tc.tile_pool(name="qk", bufs=4) as qkp,
                    tc.tile_pool(name="pt", bufs=TB + 2) as ptp,
                    tc.tile_pool(name="rl", bufs=2) as rlp,
                    tc.tile_pool(name="psqk", bufs=2, space="PSUM") as psqk,
                    tc.tile_pool(name="pss", bufs=2, space="PSUM") as pss,
                    tc.tile_pool(name="psa", bufs=2, space="PSUM") as psa,
                    tc.tile_pool(name="psl", bufs=2, space="PSUM") as psl,
                ):
                    def emit_qk(h, qk_tiles):
                        """Generator: one A1 psum-group per next() call."""
                        wqt = wqkp.tile([P, CB, 2 * d], BF16, tag="wqk")
                        nc.sync.dma_start(wqt[:], wqk[:, :, h, :])
                        qT = qkp.tile([P, T], BF16, tag="qT")
                        kT = qkp.tile([P, T], BF16, tag="kT")
                        qk_tiles[h] = (qT, kT)
                        for mi, dst in ((0, qT), (1, kT)):
                            for ho, hw in _chunks(T, SG):
                                ps = psqk.tile([P, SG], F32, tag="qk")
                                for c in range(CB):
                                    nc.tensor.matmul(
                                        ps[:, :hw],
                                        wqt[:, c, mi * d : (mi + 1) * d],
                                        xts[c][:, ho : ho + hw],
                                        start=(c == 0),
                                        stop=(c == CB - 1),
                                    )
                                nc.vector.tensor_copy(
                                    dst[:, ho : ho + hw], ps[:, :hw]
                                )
                                if (ho // SG) % 2 == 1:
                                    yield

                    def emit_strip(h, q4, qk_tiles, attnT):
                        qT, kT = qk_tiles[h]
                        s0 = q4 * QW  # strip cols [s0, s1)
                        s1 = s0 + QW
                        njs = s1 // P  # j blocks feeding this strip
                        pa = psa.tile([P, QW], F32, tag="pa")
                        pl = psl.tile([P, QW], F32, tag="pl")
                        pts = []
                        for j in range(njs):
                            c0 = max(j * P, s0)
                            w = s1 - c0
                            ps = pss.tile([P, QW], F32, tag="st")
                            nc.tensor.matmul(
                                ps[:, :w],
                                kT[:, j * P : (j + 1) * P],
                                qT[:, c0:s1],
                                start=True,
                                stop=True,
                            )
                            if j * P >= s0:  # diagonal block lives here
                                nc.vector.tensor_add(
                                    ps[:, 0:P], ps[:, 0:P], cmask[:]
                                )
                            pt = ptp.tile([P, QW], BF16, tag="pt")
                            nc.scalar.activation(
                                pt[:, :w],
                                ps[:, :w],
                                mybir.ActivationFunctionType.Exp,
                                scale=scale,
                            )
                            pts.append((pt, c0, w))
                        for j, (pt, c0, w) in enumerate(pts):
                            nc.tensor.matmul(
                                pa[:, c0 - s0 : s1 - s0],
                                Vs[j][:, h * d : (h + 1) * d],
                                pt[:, :w],
                                start=(j == 0),
                                stop=(j == njs - 1),
                                skip_group_check=True,
                            )
                        for j, (pt, c0, w) in enumerate(pts):
                            nc.tensor.matmul(
                                pl[:, c0 - s0 : s1 - s0],
                                ones[:],
                                pt[:, :w],
                                start=(j == 0),
                                stop=(j == njs - 1),
                                skip_group_check=True,
                            )
                        rl = rlp.tile([P, QW], F32, tag="rl")
                        nc.vector.reciprocal(rl[:], pl[:])
                        nc.vector.tensor_tensor(
                            out=attnT[:, s0:s1],
                            in0=pa[:],
                            in1=rl[:],
                            op=mybir.AluOpType.mult,
                        )

                    # software pipeline: head h's strips interleave with
                    # head h+1's qk psum-groups to fill each other's stalls
                    qk_tiles = {}
                    n_strips = T // QW
                    for _ in emit_qk(0, qk_tiles):
                        pass
                    for h in range(HL):
                        nxt = (
                            emit_qk(h + 1, qk_tiles) if h + 1 < HL else iter(())
                        )
                        attnT = attnp.tile([P, T], BF16, tag="attnT")
                        for q4 in range(n_strips):
                            emit_strip(h, q4, qk_tiles, attnT)
                            next(nxt, None)
                        for _ in nxt:
                            pass
                        attnTs.append(attnT)
                        qk_tiles.pop(h, None)

            # ---------------- Phase C: y = attn @ Wp.T (partial) --------
            with (
                tc.tile_pool(name="wp", bufs=CH // P) as wpp,
                tc.tile_pool(name="ysb", bufs=2) as ysbp,
                tc.tile_pool(name="psy", bufs=2, space="PSUM") as psy,
            ):
                wpts = []
                for ch in range(CH // P):
                    wpt = wpp.tile([P, C], BF16, tag="wp")
                    nc.sync.dma_start(wpt[:], wp[ch * P : (ch + 1) * P, :])
                    wpts.append(wpt)
                for tb in range(TB):
                    ysb = ysbp.tile([P, C], F32, tag="ysb")
                    for ho, hw in _chunks(C, YH):
                        ps = psy.tile([P, YH], F32, tag="y")
                        for ch in range(CH // P):
                            hd = ch * P // d  # owning local head of this chunk
                            for o, w in _chunks(hw, SG):
                                nc.tensor.matmul(
                                    ps[:, o : o + w],
                                    attnTs[hd][:, tb * P : (tb + 1) * P],
                                    wpts[ch][:, ho + o : ho + o + w],
                                    start=(ch == 0),
                                    stop=(ch == CH // P - 1),
                                )
                        nc.vector.tensor_copy(ysb[:, ho : ho + hw], ps[:, :hw])
                    nc.sync.dma_start(y[tb * P : (tb + 1) * P, :], ysb[:])

    return nc


def make_core_inputs(x, W_attn, W_proj, b, g, T=2048, C=2048, HL=8, d=128):
    """Host-side shard + layout prep for core (batch b, head-group g)."""
    bf16 = ml_dtypes.bfloat16
    CB = C // P
    CH = HL * d
    xb = np.asarray(x[b], dtype=np.float32)  # [T, C]
    xT = np.ascontiguousarray(xb.T).astype(bf16)  # [C, T]

    q_rows = W_attn[g * CH : (g + 1) * CH, :]  # [CH, C]
    k_rows = W_attn[C + g * CH : C + (g + 1) * CH, :]
    v_rows = W_attn[2 * C + g * CH : 2 * C + (g + 1) * CH, :]

    # wqk[p, cc, h, col]: col 0:d -> WqT, d:2d -> WkT; c = cc*128 + p
    wqkT = np.empty((C, HL, 2 * d), dtype=np.float32)
    for h in range(HL):
        wqkT[:, h, :d] = q_rows[h * d : (h + 1) * d, :].T
        wqkT[:, h, d:] = k_rows[h * d : (h + 1) * d, :].T
    wqk = np.ascontiguousarray(
        wqkT.reshape(CB, P, HL, 2 * d).transpose(1, 0, 2, 3)
    ).astype(bf16)

    wv = np.ascontiguousarray(v_rows.T).astype(bf16)  # [C, CH]
    wpm = np.ascontiguousarray(W_proj[:, g * CH : (g + 1) * CH].T).astype(
        bf16
    )  # [CH, C]
    return {"xT": xT, "wqk": wqk, "wv": wv, "wp": wpm}


def core_reference(xT, wqk, wv, wp, T=2048, C=2048, HL=8, d=128):
    """Numpy replica of one core's program (bf16 inputs, fp32 accum)."""
    CB = C // P
    xTf = np.asarray(xT, dtype=np.float32)
    x = xTf.T  # [T, C]
    wqkf = np.asarray(wqk, dtype=np.float32).transpose(1, 0, 2, 3).reshape(C, HL, 2 * d)
    V = x @ np.asarray(wv, dtype=np.float32)  # [T, CH]
    out = np.zeros((T, HL * d), dtype=np.float32)
    for h in range(HL):
        q = x @ wqkf[:, h, :d]  # [T, d]
        k = x @ wqkf[:, h, d:]
        S = (q @ k.T) / math.sqrt(d)
        mask = np.triu(np.ones((T, T), dtype=bool), 1)
        S = np.where(mask, -np.inf, S)
        Pm = np.exp(S)
        Pm = Pm / Pm.sum(-1, keepdims=True)
        out[:, h * d : (h + 1) * d] = Pm @ V[:, h * d : (h + 1) * d]
    return out @ np.asarray(wp, dtype=np.float32)  # [T, C] partial


_CACHE = {}


def _get_nc():
    if "nc" not in _CACHE:
        nc = build_nc()
        nc.compile()
        _CACHE["nc"] = nc
    return _CACHE["nc"]


def run_cores(in_maps, trace=False):
    from concourse.bass_utils import run_bass_kernel_spmd

    nc = _get_nc()
    return run_bass_kernel_spmd(nc, in_maps, list(range(len(in_maps))), trace=trace)


def kernel(x, W_attn, W_proj):
    x = np.asarray(x, dtype=np.float32)
    W_attn = np.asarray(W_attn, dtype=np.float32)
    W_proj = np.asarray(W_proj, dtype=np.float32)
    B = x.shape[0]
    in_maps = [make_core_inputs(x, W_attn, W_proj, i // 2, i % 2) for i in range(8)]
    res = run_cores(in_maps).results
    y = np.stack(
        [res[2 * b]["y"].astype(np.float32) + res[2 * b + 1]["y"] for b in range(B)]
    )
    return y
